# revision 17
# baseline (speedup 1.0000x reference)
"""Trainium2 Bass kernel for nn_CrossAttentionLayer (B=4, C=256, H=W=64).

Sharding: 8 cores = batch(4) x query-half(2). Each core computes a
[C, N/2] = [256, 2048] output shard from x1-half [256, 2048] and full
x2 [256, 4096] for its batch. BN / conv-bias / 1/sqrt(d) are folded into
the weights host-side. All big matmuls run in float32r (TF32-like full
rate on the PE); softmax skips the max-subtraction (energies are small,
fp32 exp is exact enough) and its normalization is folded to the end:

  out[c, m] = (sum_n v[c, n] * exp(S[m, n])) / (sum_n exp(S[m, n])) + bv[c]
"""

import numpy as np
from contextlib import ExitStack, nullcontext

import concourse.bass as bass
import concourse.bacc as bacc
import concourse.mybir as mybir
import concourse.tile as tile
from concourse.bass_utils import run_bass_kernel_spmd

dt = mybir.dt
F32, F32R = dt.float32, dt.float32r
EPS = 1e-5
B, C, Hs, Ws = 4, 256, 64, 64
N = Hs * Ws            # 4096 spatial positions
DQK = C // 8           # 32
NCORES = 8
MH = N // 2            # 2048 query rows per core
NT = N // 128          # 32 key-side n-tiles
MQS = 512              # m-chunk (one PSUM bank of fp32)
MQ = MH // MQS         # 4 m-chunks

_prog = None
LAST_RESULTS = None
LAST_IN_MAPS = None


def _build(reps=1):
    nc = bacc.Bacc("TRN2", target_bir_lowering=False, debug=False)
    x1s = nc.dram_tensor("x1s", [C, MH], F32, kind="ExternalInput")
    x2s = nc.dram_tensor("x2s", [C, N], F32, kind="ExternalInput")
    # wqk: [128, 2k+{0:q,1:k}*32+d] ; wv: [128, 2k*256+c] ; bias: bq|bk|bv0|bv1
    wqk = nc.dram_tensor("wqk", [128, 2 * 2 * DQK], F32, kind="ExternalInput")
    wvd = nc.dram_tensor("wv", [128, 2 * C], F32, kind="ExternalInput")
    bias = nc.dram_tensor("bias", [128, 4], F32, kind="ExternalInput")
    y = nc.dram_tensor("y", [C, MH], F32, kind="ExternalOutput")

    Exp = mybir.ActivationFunctionType.Exp

    with tile.TileContext(nc) as tc, ExitStack() as ctx:
        sbc = ctx.enter_context(tc.tile_pool(name="sbc", bufs=1))
        sbx = ctx.enter_context(tc.tile_pool(name="sbx", bufs=3))
        sbp = ctx.enter_context(tc.tile_pool(name="sbp", bufs=3))
        psa = ctx.enter_context(tc.tile_pool(name="psa", bufs=3, space="PSUM"))
        pso = ctx.enter_context(tc.tile_pool(name="pso", bufs=4, space="PSUM"))
        psr = ctx.enter_context(tc.tile_pool(name="psr", bufs=1, space="PSUM"))
        if reps > 1:  # benchmarking: repeat the whole body on-device
            ctx.enter_context(tc.For_i(
                0, reps, 1,
                hint_engines=(mybir.EngineType.PE, mybir.EngineType.Activation,
                              mybir.EngineType.DVE, mybir.EngineType.SP),
            ))

        # weights / biases: 3 packed DMAs, rounded to f32r
        wqk_raw = sbx.tile([128, 2 * 2 * DQK], F32, tag="wqkraw")
        nc.sync.dma_start(wqk_raw[:], wqk.ap())
        wqk_r = sbc.tile([128, 2 * 2 * DQK], F32R, tag="wqk_r")
        nc.vector.tensor_copy(wqk_r[:], wqk_raw[:])
        wq_r = [wqk_r[:, (2 * k + 0) * DQK:(2 * k + 1) * DQK] for k in range(2)]
        wk_r = [wqk_r[:, (2 * k + 1) * DQK:(2 * k + 2) * DQK] for k in range(2)]

        wv_raw = sbx.tile([128, 2 * C], F32, tag="wvraw")
        nc.sync.dma_start(wv_raw[:], wvd.ap())
        wv_rt = sbc.tile([128, 2 * C], F32R, tag="wv_r")
        nc.vector.tensor_copy(wv_rt[:], wv_raw[:])
        wv_r = [wv_rt[:, k * C:(k + 1) * C] for k in range(2)]

        bias_t = sbc.tile([128, 4], F32, tag="bias_t")
        nc.sync.dma_start(bias_t[:], bias.ap())
        bq_t = bias_t[0:DQK, 0:1]
        bk_t = bias_t[0:DQK, 1:2]
        bv_t = [bias_t[:, 2 + i:3 + i] for i in range(2)]

        ones_f = sbc.tile([128, 1], F32, tag="ones_f")
        nc.vector.memset(ones_f[:], 1.0)
        ones_r = sbc.tile([128, 1], F32R, tag="ones_r")
        nc.vector.tensor_copy(ones_r[:], ones_f[:])

        # activations -> SBUF in 512-col chunks, rounded to f32r, with the
        # K'/Q'/V^T projections pipelined chunk-by-chunk behind the DMAs.
        ksb = sbc.tile([DQK, N], F32R, tag="ksb")
        qsb = sbc.tile([DQK, MH], F32R, tag="qsb")
        vt = sbc.tile([128, NT, C], F32R, tag="vt")
        x2_r = [sbx.tile([128, N], F32R, tag=f"x2r{k}", name=f"x2r{k}", bufs=1)
                for k in range(2)]
        x1_r = [sbx.tile([128, MH], F32R, tag=f"x1r{k}", name=f"x1r{k}", bufs=1)
                for k in range(2)]

        for cch in range(N // 512):
            csl = slice(cch * 512, (cch + 1) * 512)
            for k in range(2):
                raw = sbx.tile([128, 512], F32, tag="xraw", name=f"x2raw_{cch}_{k}")
                nc.sync.dma_start(raw[:], x2s.ap()[k * 128:(k + 1) * 128, csl])
                nc.vector.tensor_copy(x2_r[k][:, csl], raw[:])
            # K' = fold(Wk) @ x2 + bk   -> [32, N] f32r
            pk = psa.tile([DQK, 512], F32, tag="st", name=f"pk{cch}")
            for k in range(2):
                nc.tensor.matmul(pk[:], wk_r[k][:], x2_r[k][:, csl],
                                 start=(k == 0), stop=(k == 1))
            nc.vector.tensor_scalar_add(ksb[:, csl], pk[:], bk_t[:])
            # V^T tiles: vt[:, nt, c] = (x2^T Wv^T)[n, c]  (no bias)
            for nt in range(4 * cch, 4 * cch + 4):
                pv = psa.tile([128, C], F32, tag="st", name=f"pv{nt}")
                for k in range(2):
                    nc.tensor.matmul(pv[:], x2_r[k][:, nt * 128:(nt + 1) * 128],
                                     wv_r[k][:], start=(k == 0), stop=(k == 1))
                nc.vector.tensor_copy(vt[:, nt, :], pv[:])

        for cch in range(MH // 512):
            csl = slice(cch * 512, (cch + 1) * 512)
            for k in range(2):
                raw = sbx.tile([128, 512], F32, tag="xraw", name=f"x1raw_{cch}_{k}")
                nc.sync.dma_start(raw[:], x1s.ap()[k * 128:(k + 1) * 128, csl])
                nc.vector.tensor_copy(x1_r[k][:, csl], raw[:])
            # Q' = fold(Wq) @ x1_half + bq  (scale folded) -> [32, MH] f32r
            pq = psa.tile([DQK, 512], F32, tag="st", name=f"pq{cch}")
            for k in range(2):
                nc.tensor.matmul(pq[:], wq_r[k][:], x1_r[k][:, csl],
                                 start=(k == 0), stop=(k == 1))
            nc.vector.tensor_scalar_add(qsb[:, csl], pq[:], bq_t[:])

        # main loop: S^T tiles -> exp -> PV accumulate (+rowsum), software-
        # pipelined so the PE issues ST(i+1) while ACT runs exp(i) and the
        # PV matmuls for step i wait only on an already-finished exp.
        oc_t, rs_t, pt_t = {}, {}, {}

        def emit_st(mq, nt):
            msl = slice(mq * MQS, (mq + 1) * MQS)
            if nt == 0:
                oc_t[mq] = [pso.tile([128, MQS], F32, tag="outc",
                                     name=f"oc{mq}_{i}") for i in range(2)]
                rs_t[mq] = psr.tile([1, MQS], F32, tag="rowsum", name=f"rs{mq}")
            st = psa.tile([128, MQS], F32, tag="st", name=f"st{mq}_{nt}")
            nc.tensor.matmul(st[:], ksb[:, nt * 128:(nt + 1) * 128], qsb[:, msl],
                             start=True, stop=True)
            pt = sbp.tile([128, MQS], F32R, tag="pt", name=f"pt{mq}_{nt}", bufs=4)
            nc.scalar.activation(pt[:], st[:], Exp)
            pt_t[(mq, nt)] = pt

        def emit_pv(mq, nt):
            pt = pt_t.pop((mq, nt))
            first, last = nt == 0, nt == NT - 1
            for ci in range(2):
                nc.tensor.matmul(oc_t[mq][ci][:], vt[:, nt, ci * 128:(ci + 1) * 128],
                                 pt[:], start=first, stop=last)
            nc.tensor.matmul(rs_t[mq][:], ones_r[:], pt[:], start=first, stop=last)
            if last:
                emit_finalize(mq)

        def emit_finalize(mq):
            msl = slice(mq * MQS, (mq + 1) * MQS)
            recip = sbp.tile([1, MQS], F32, tag="recip", name=f"recip{mq}")
            nc.vector.reciprocal(recip[:], rs_t[mq][:])
            recipb = sbp.tile([128, MQS], F32, tag="recipb", name=f"recipb{mq}")
            nc.gpsimd.partition_broadcast(recipb[:], recip[:])
            for ci in range(2):
                yt = sbp.tile([128, MQS], F32, tag="y", name=f"y{mq}_{ci}")
                nc.vector.tensor_mul(yt[:], oc_t[mq][ci][:], recipb[:])
                nc.vector.tensor_scalar_add(yt[:], yt[:], bv_t[ci][:])
                nc.sync.dma_start(y.ap()[ci * 128:(ci + 1) * 128, msl], yt[:])

        steps = [(mq, nt) for mq in range(MQ) for nt in range(NT)]
        DEPTH = 2  # ST runs this many steps ahead of its PV consumers
        for i in range(DEPTH):
            emit_st(*steps[i])
        for i in range(DEPTH, len(steps)):
            emit_st(*steps[i])
            emit_pv(*steps[i - DEPTH])
        for i in range(len(steps) - DEPTH, len(steps)):
            emit_pv(*steps[i])

    nc.compile()
    return nc


def _fold_weights(w, b, gamma, beta, mean, var, scale=1.0):
    w = w.astype(np.float64)
    inv = gamma.astype(np.float64) / np.sqrt(var.astype(np.float64) + EPS)
    shift = beta.astype(np.float64) - mean.astype(np.float64) * inv
    wf = w * inv[:, None] * scale
    bf = (b.astype(np.float64) * inv + shift) * scale
    return (np.ascontiguousarray(wf.T).astype(np.float32),
            bf.astype(np.float32)[:, None])


def kernel(x1, x2, q_w, q_b, q_gamma, q_beta, q_mean, q_var,
           k_w, k_b, k_gamma, k_beta, k_mean, k_var,
           v_w, v_b, v_gamma, v_beta, v_mean, v_var):
    global _prog, LAST_RESULTS
    if _prog is None:
        _prog = _build()

    s = 1.0 / np.sqrt(np.float64(DQK))
    WqT, bq = _fold_weights(q_w, q_b, q_gamma, q_beta, q_mean, q_var, s)
    WkT, bk = _fold_weights(k_w, k_b, k_gamma, k_beta, k_mean, k_var)
    WvT, bv = _fold_weights(v_w, v_b, v_gamma, v_beta, v_mean, v_var)

    # pack small weights into wide [128, x] tensors for efficient DMA
    wqk_pack = np.zeros((128, 2 * 2 * DQK), np.float32)
    wv_pack = np.zeros((128, 2 * C), np.float32)
    for k in range(2):
        wqk_pack[:, (2 * k + 0) * DQK:(2 * k + 1) * DQK] = WqT[128 * k:128 * (k + 1)]
        wqk_pack[:, (2 * k + 1) * DQK:(2 * k + 2) * DQK] = WkT[128 * k:128 * (k + 1)]
        wv_pack[:, k * C:(k + 1) * C] = WvT[128 * k:128 * (k + 1)]
    bias_pack = np.zeros((128, 4), np.float32)
    bias_pack[:DQK, 0] = bq[:, 0]
    bias_pack[:DQK, 1] = bk[:, 0]
    bias_pack[:, 2] = bv[0:128, 0]
    bias_pack[:, 3] = bv[128:256, 0]

    x1f = np.asarray(x1, dtype=np.float32).reshape(B, C, N)
    x2f = np.asarray(x2, dtype=np.float32).reshape(B, C, N)

    in_maps = []
    for core in range(NCORES):
        b, h = divmod(core, 2)
        in_maps.append({
            "x1s": np.ascontiguousarray(x1f[b][:, h * MH:(h + 1) * MH]),
            "x2s": np.ascontiguousarray(x2f[b]),
            "wqk": wqk_pack, "wv": wv_pack, "bias": bias_pack,
        })

    global LAST_IN_MAPS
    LAST_IN_MAPS = in_maps
    LAST_RESULTS = run_bass_kernel_spmd(_prog, in_maps, core_ids=list(range(NCORES)))
    out = np.empty((B, C, N), np.float32)
    for core in range(NCORES):
        b, h = divmod(core, 2)
        out[b][:, h * MH:(h + 1) * MH] = LAST_RESULTS.results[core]["y"]
    return out.reshape(B, C, Hs, Ws)
